# revision 15
# baseline (speedup 1.0000x reference)
"""Trainium2 Bass kernel for masked-softmax attention (sparse_attention).

reference:
    S = Q @ K^T / sqrt(128)            # [N, nq, nk]
    A = softmax(S, axis=-1) * mask
    A = A / (sum_k A + 1e-6)
    O = A @ V

Identity used on-device (softmax normalizer cancels in the renormalization):
    E = exp(S); P = E * mask
    O[q, :] = (P @ V)[q, :] / (sum_k P[q, k] + eps * sum_k E[q, k])
The eps*Z term is ~2e-6 relative to the masked sum (mask ~ U[0,1), nk=2048) and
is dropped.

Sharding: N=32 batch-heads split across 8 NeuronCores, 4 per core. No
cross-core communication.

Per-core pipeline, per batch b and 128-row q-tile:
  mm1   (PE, f32r):  S-tile = QT.T @ KT                  -> PSUM
  exp   (ACT):       E = exp(S/sqrt(d)) [bf16]           -> SBUF
  mult  (DVE, bf16): P = E * mask  (mask cast-DMA'd bf16) -> SBUF
  trans (PE, bf16):  P^T tiles (k on partitions)          -> PSUM
  copy  (DVE):       P^T -> SBUF
  mm2   (PE, bf16):  O|denom = P^T.T @ [V | ones]         -> PSUM (accum 16 k-tiles)
  recip+scale (DVE): out = O * (1/denom)                  -> SBUF -> DMA out
"""
import sys

sys.path.insert(0, "/opt/trn_rl_repo")

import numpy as np

from concourse import bacc, mybir, tile
from concourse.bass_utils import run_bass_kernel_spmd

N, NQ, NK, D = 32, 2048, 2048, 128
N_CORES = 8
B = N // N_CORES          # batches per core
QT_TILES = NQ // 128      # 16 q tiles per batch
KT_TILES = NK // 128      # 16 k tiles per batch
SCALE = float(1.0 / np.sqrt(D))
MASK_CHUNK = 2            # q-tiles of mask per DMA (2 * 1MB f32 read)

F32 = mybir.dt.float32
F32R = mybir.dt.float32r
BF16 = mybir.dt.bfloat16

_cached = {}


def build():
    if "nc" in _cached:
        return _cached["nc"]
    nc = bacc.Bacc("TRN2", target_bir_lowering=False, debug=False)

    qt_d = nc.dram_tensor("queriesT", [B, D, NQ], F32, kind="ExternalInput").ap()
    kt_d = nc.dram_tensor("keysT", [B, D, NK], F32, kind="ExternalInput").ap()
    v_d = nc.dram_tensor("valuesP", [B, 128, KT_TILES, D], F32, kind="ExternalInput").ap()
    m_d = nc.dram_tensor("mask", [B, NQ, NK], F32, kind="ExternalInput").ap()
    id_d = nc.dram_tensor("ident", [128, 128], F32, kind="ExternalInput").ap()
    o_d = nc.dram_tensor("out", [B, 128, QT_TILES, D], BF16, kind="ExternalOutput").ap()

    with tile.TileContext(nc) as tc:
        with (
            tc.tile_pool(name="const", bufs=1) as cpool,
            tc.tile_pool(name="nat", bufs=2) as natpool,
            tc.tile_pool(name="tr", bufs=2) as trpool,
            tc.tile_pool(name="vbo", bufs=2) as vpool,
            tc.tile_pool(name="maskc", bufs=6) as mpool,
            tc.tile_pool(name="work", bufs=4) as wpool,
            tc.tile_pool(name="stage", bufs=2) as stpool,
            tc.tile_pool(name="spsum", bufs=2, space="PSUM") as spool,
            tc.tile_pool(name="ptpsum", bufs=2, space="PSUM") as ptpool,
            tc.tile_pool(name="opsum", bufs=2, space="PSUM") as opool,
        ):
            ident_b = cpool.tile([128, 128], BF16, tag="identb")
            nc.gpsimd.dma_start(out=ident_b[:], in_=id_d)

            for b in range(B):
                # ---- per-batch prep: load QT/KT (host-pretransposed, f32r),
                # V natural, build V|1 (bf16)
                kt_sb = trpool.tile([128, NK], F32R, tag="kt")
                nc.sync.dma_start(kt_sb[:], kt_d[b].bitcast(F32R))
                qt_sb = trpool.tile([128, NQ], F32R, tag="qt")
                for qq in range(4):
                    nc.sync.dma_start(
                        qt_sb[:, qq * (NQ // 4):(qq + 1) * (NQ // 4)],
                        qt_d[b, :, qq * (NQ // 4):(qq + 1) * (NQ // 4)].bitcast(F32R),
                    )
                vnb = natpool.tile([128, KT_TILES, D], BF16, tag="vn")
                nc.gpsimd.dma_start(out=vnb[:], in_=v_d[b])
                vb = vpool.tile([128, KT_TILES, D + 1], BF16, tag="vb")
                nc.vector.tensor_copy(vb[:, :, 0:D], vnb[:])
                nc.vector.memset(vb[:, :, D], 1.0)

                st = stpool.tile([128, QT_TILES, D], BF16, tag="st")

                for qt in range(QT_TILES):
                    if qt % MASK_CHUNK == 0:
                        mask_c = mpool.tile([128, MASK_CHUNK, NK], BF16, tag="mc")
                        nc.gpsimd.dma_start(
                            out=mask_c[:],
                            in_=m_d[b, qt * 128:(qt + MASK_CHUNK) * 128, :].rearrange(
                                "(g p) k -> p g k", p=128
                            ),
                        )

                    # mm1 + exp, in k-halves of 1024
                    e_sb = wpool.tile([128, NK], BF16, tag="e")
                    for h in range(2):
                        s_ps = spool.tile([128, 1024], F32, tag="s")
                        for c in range(2):
                            nc.tensor.matmul(
                                s_ps[:, c * 512:(c + 1) * 512],
                                qt_sb[:, qt * 128:(qt + 1) * 128],
                                kt_sb[:, h * 1024 + c * 512: h * 1024 + (c + 1) * 512],
                                start=True,
                                stop=True,
                            )
                        nc.scalar.activation(
                            e_sb[:, h * 1024:(h + 1) * 1024],
                            s_ps[:],
                            mybir.ActivationFunctionType.Exp,
                            scale=SCALE,
                        )

                    # P = E * mask (bf16, 2x mode)
                    p_sb = wpool.tile([128, NK], BF16, tag="p")
                    nc.vector.tensor_tensor(
                        out=p_sb[:],
                        in0=e_sb[:],
                        in1=mask_c[:, qt % MASK_CHUNK, :],
                        op=mybir.AluOpType.mult,
                    )

                    # transpose P -> P^T tiles (bf16) in PSUM, copy to SBUF
                    # (two half-tiles for finer pipelining; copies split
                    # DVE/ACT to balance engine load)
                    pt_sb = wpool.tile([128, NK], BF16, tag="pt")
                    for h in range(2):
                        pt_ps = ptpool.tile([128, NK // 2], BF16, tag="ptps")
                        for t in range(KT_TILES // 2):
                            tt = h * (KT_TILES // 2) + t
                            nc.tensor.transpose(
                                pt_ps[:, t * 128:(t + 1) * 128],
                                p_sb[:, tt * 128:(tt + 1) * 128],
                                ident_b[:],
                            )
                        dst = pt_sb[:, h * (NK // 2):(h + 1) * (NK // 2)]
                        if (2 * qt + h) % 4 == 3:
                            nc.scalar.copy(dst, pt_ps[:])
                        else:
                            nc.vector.tensor_copy(dst, pt_ps[:])

                    # mm2: O|denom = sum_k P^T.T @ [V | 1]
                    o_ps = opool.tile([128, D + 1], F32, tag="o")
                    for t in range(KT_TILES):
                        nc.tensor.matmul(
                            o_ps[:],
                            pt_sb[:, t * 128:(t + 1) * 128],
                            vb[:, t, :],
                            start=(t == 0),
                            stop=(t == KT_TILES - 1),
                        )

                    rd = wpool.tile([128, 1], F32, tag="rd")
                    nc.vector.reciprocal(rd[:], o_ps[:, D:D + 1])
                    nc.vector.tensor_scalar_mul(st[:, qt, :], o_ps[:, 0:D], rd[:])

                # output store on the ACT HWDGE ring so it doesn't queue in
                # front of the next batch's input loads on the sync ring
                nc.scalar.dma_start(o_d[b], st[:])

    nc.compile()
    _cached["nc"] = nc
    return nc


def kernel(queries, keys, values, mask, _trace=False, **kw):
    nc = build()
    ident = np.eye(128, dtype=np.float32)
    in_maps = []
    for c in range(N_CORES):
        sl = slice(c * B, (c + 1) * B)
        in_maps.append(
            {
                "queriesT": np.ascontiguousarray(queries[sl].transpose(0, 2, 1)),
                "keysT": np.ascontiguousarray(keys[sl].transpose(0, 2, 1)),
                "valuesP": np.ascontiguousarray(
                    values[sl].reshape(B, KT_TILES, 128, D).transpose(0, 2, 1, 3)
                ),
                "mask": np.ascontiguousarray(mask[sl]),
                "ident": ident,
            }
        )
    res = run_bass_kernel_spmd(
        nc, in_maps, core_ids=list(range(N_CORES)), trace=_trace
    )
    out = np.concatenate(
        [
            res.results[c]["out"].astype(np.float32).transpose(0, 2, 1, 3).reshape(B, NQ, D)
            for c in range(N_CORES)
        ],
        axis=0,
    )
    if _trace:
        return out.astype(np.float32, copy=False), res
    return out.astype(np.float32, copy=False)
